# revision 10
# baseline (speedup 1.0000x reference)
"""Trainium2 Bass kernel for nn_BiLSTM_M_61615600828569 (segment_reduce).

Full computation per batch:
  span_emb = masked-max-pool of token windows   (B,256,768)
  vertex_emb = masked-mean over coref spans     (B,128,768)
  head/tail  = vertex gather by relation        (B,512,768)
  feat = [head, eh, tail, et, head*tail]        (B,512,2344)
  out  = relu(feat @ W1) @ W2 + b2              (B,512,97)

Sharding: data-parallel over batch; 16 batches / 8 cores = 2 per core.
All index work (gather tables, one-hot select matrices, pooling weights)
is precomputed on host; all float math runs on device in bf16 with fp32
PSUM accumulation, in transposed layout (features on partitions) so the
final predict.T has the 97 classes on partitions for a per-partition
bias add.

Span pooling: spans are sorted by width per batch (the permutation is
folded into the host-built pool matrix, so it is free).  The widest 128
spans are fetched as single 8-row dma_gather descriptors, the narrowest
128 as 5-row descriptors (valid unless >128 spans have width>=5, which
triggers a recompile with an 8-row fallback).  Rows past each span's
width are killed inside a depth-3 max tree whose leaves add a
per-partition -2e30 mask before each pairwise max.

DMA schedule: only tiny tables are loaded up front so the gathers own
the HBM bandwidth; the big weight/select loads are issued from the
Vector stream after batch-0's max tree, landing just before their
consumers.  dis_embed@W1 blocks are folded on host into one 40-row
contraction; V_emb.T comes from PE transposes instead of matmuls.
"""
import numpy as np
import ml_dtypes
from contextlib import ExitStack

import concourse.bass as bass
import concourse.bacc as bacc
import concourse.tile as tile
from concourse import mybir
from concourse import bass_utils

BF16 = ml_dtypes.bfloat16

B, S, D = 16, 1024, 768
NS, MAXW = 256, 8
V, C = 128, 6
R = 512
REL, HID, DIS = 97, 384, 20
NEG = -2e30

NCORES = 8
NB = B // NCORES          # batches per core = 2
SENT_ROWS = NB * S
NW1C = 18                 # W1 chunks: a(6) + c(6) + e(6)
W2C = HID // 128          # 3

# bf16 "small" pack layout (columns)
PK_ID = 0                 # identity [128,128]
PK_PT = 128               # poolt    [128, NB*2*V=512]
PK_EW = 640               # ew2      [40, 384]
PK_ES = 1024              # esel2    [40, NB*R=1024]
PK_N = 2048
# f32 pack layout (columns)
FK_MASK = 0               # masks [128, NB*2*7=28]
FK_INV = 28               # invcnt [128, NB]
FK_B2 = 30                # b2 [97, 1]
FK_N = 31
# w1/w2 pack (bf16) columns
WK_W1 = 0                 # [128, 18*384]
WK_W2 = NW1C * 384        # [128, 3*97]
WK_N = WK_W2 + W2C * REL


def _patch_drain_and_barrier():
    """Walrus rejects >1 explicit sync wait on a Drain (TPB_CTRL), but Tile's
    tail drain waits on every used proc sem at once. Emit one single-wait
    drain per proc instead; the final drain then needs no waits."""
    import concourse.tile as tile_mod
    from concourse.vector_clock import VectorClock, ScopedClock

    if getattr(tile_mod.TileContext, "_ant_drain_patched", False):
        return

    def _patched(self, tick_clock, wait_clock):
        full = tick_clock.global_clock
        n = len(full)
        engines = [self.nc.sync, self.nc.vector, self.nc.scalar,
                   self.nc.tensor, self.nc.gpsimd]
        for i, p in enumerate([q for q in range(n) if full[q] > 0]):
            vec = [full[q] if q == p else 0 for q in range(n)]
            d = engines[i % len(engines)].drain()
            wait_clock.add_sem_waits(d.ins, ScopedClock({None: VectorClock(vec)}))
        self.nc.sync.drain()
        self.nc.all_engine_barrier()
        popped = self.nc._tile_sem_poison_stack.pop()
        assert popped is self._sem_poison
        self.nc.clear_and_free_semaphores(list(self.sems.allocated().values()))
        self.nc.all_engine_barrier()

    tile_mod.TileContext._drain_and_barrier = _patched
    tile_mod.TileContext._ant_drain_patched = True


_patch_drain_and_barrier()

_NC_CACHE = {}


def _build(cc1_rows):
    """One-core program; SPMD-replicated across the 8 cores.
    cc1_rows: rows fetched per narrow-span descriptor (5, or 8 fallback)."""
    bf = mybir.dt.bfloat16
    f32 = mybir.dt.float32
    AF = mybir.ActivationFunctionType
    ADD = mybir.AluOpType.add
    MAX = mybir.AluOpType.max
    MUL = mybir.AluOpType.mult

    nc = bacc.Bacc("TRN2", target_bir_lowering=False, debug=False, num_devices=1)

    sent = nc.dram_tensor("sent", (SENT_ROWS, D), bf, kind="ExternalInput")
    gidx = nc.dram_tensor("gidx", (128, NB * 2 * 8), mybir.dt.int16, kind="ExternalInput")
    pkf = nc.dram_tensor("pkf", (128, FK_N), f32, kind="ExternalInput")
    pkb = nc.dram_tensor("pkb", (128, PK_N), bf, kind="ExternalInput")
    hts = nc.dram_tensor("hts", (V, 2 * NB * R), bf, kind="ExternalInput")
    wpk = nc.dram_tensor("wpk", (128, WK_N), bf, kind="ExternalInput")
    outd = nc.dram_tensor("outd", (REL, NB, R), bf, kind="ExternalOutput")

    # overlapping row-window views of the staged sentence
    def sent_view(nrows):
        return bass.AP(tensor=sent.ap().tensor, offset=0,
                       ap=[[D, SENT_ROWS - (nrows - 1)], [1, nrows * D]])

    with tile.TileContext(nc) as tc, ExitStack() as ctx:
        consts = ctx.enter_context(tc.tile_pool(name="consts", bufs=1))
        work = ctx.enter_context(tc.tile_pool(name="work", bufs=1))
        perb = ctx.enter_context(tc.tile_pool(name="perb", bufs=2))
        psums = ctx.enter_context(tc.tile_pool(name="psums", bufs=1, space="PSUM"))

        def psum_tile(name, tag, bufs, fdim=R):
            return psums.tile([128, fdim], mybir.dt.float32, space="PSUM",
                              tag=tag, bufs=bufs, name=name)

        # ---- tiny early loads on sync: gather indices, f32 masks, bf16 pack
        idx_t = consts.tile([128, NB * 2 * 8], mybir.dt.int16)
        nc.sync.dma_start(out=idx_t[:], in_=gidx.ap())
        pkf_t = consts.tile([128, FK_N], f32)
        nc.sync.dma_start(out=pkf_t[:], in_=pkf.ap())
        pkb_t = consts.tile([128, PK_N], bf)
        nc.sync.dma_start(out=pkb_t[:], in_=pkb.ap())

        # ---- span gathers: per batch h: cc0 = 8-row quads, cc1 = cc1_rows
        gw = [None] * NB
        gn = [None] * NB
        for h in range(NB):
            gwt = work.tile([128, 1, 8 * D], bf, name=f"gw_{h}", tag=f"gw_{h}")
            nc.gpsimd.dma_gather(
                out_ap=gwt[:], in_ap=sent_view(8),
                idxs_ap=idx_t[:, (2 * h) * 8 : (2 * h) * 8 + 8],
                num_idxs=128, num_idxs_reg=128,
                elem_size=8 * D, elem_step=D, single_packet=False)
            gw[h] = gwt
            gnt = work.tile([128, 1, cc1_rows * D], bf, name=f"gn_{h}", tag=f"gn_{h}")
            nc.gpsimd.dma_gather(
                out_ap=gnt[:], in_ap=sent_view(cc1_rows),
                idxs_ap=idx_t[:, (2 * h + 1) * 8 : (2 * h + 1) * 8 + 8],
                num_idxs=128, num_idxs_reg=128,
                elem_size=cc1_rows * D, elem_step=D, single_packet=False)
            gn[h] = gnt

        # big loads issued from GpSimd after the gather preps so the gathers
        # own the HBM bandwidth while the early tables load
        hts_t = consts.tile([V, 2 * NB * R], bf)
        nc.gpsimd.dma_start(out=hts_t[:], in_=hts.ap())
        wpk_t = consts.tile([128, WK_N], bf)
        nc.gpsimd.dma_start(out=wpk_t[:], in_=wpk.ap())

        # ---- max trees.  masks m[p, (h,cc,j)]: add NEG where row invalid.
        def mask(h, cc, j):
            return pkf_t[:, (h * 2 + cc) * 7 + j : (h * 2 + cc) * 7 + j + 1]

        sem_b = []  # sem_b[h] = [128, 2, D]; [:,0]=widest 128 slots, [:,1]=rest
        for h in range(NB):
            sh = work.tile([128, 2, D], bf, name=f"sem_{h}", tag=f"sem_{h}")
            g0 = gw[h][:].rearrange("p one (r d) -> p (one r) d", r=8)
            # cc0 on DVE: 7 STT, depth 3
            t01 = work.tile([128, D], bf, name=f"t01_{h}", tag=f"t01_{h}")
            t23 = work.tile([128, D], bf, name=f"t23_{h}", tag=f"t23_{h}")
            t45 = work.tile([128, D], bf, name=f"t45_{h}", tag=f"t45_{h}")
            t67 = work.tile([128, D], bf, name=f"t67_{h}", tag=f"t67_{h}")
            nc.vector.scalar_tensor_tensor(out=t01[:], in0=g0[:, 1, :], scalar=mask(h, 0, 0),
                                           in1=g0[:, 0, :], op0=ADD, op1=MAX)
            nc.vector.scalar_tensor_tensor(out=t23[:], in0=g0[:, 3, :], scalar=mask(h, 0, 1),
                                           in1=g0[:, 2, :], op0=ADD, op1=MAX)
            nc.vector.scalar_tensor_tensor(out=t45[:], in0=g0[:, 5, :], scalar=mask(h, 0, 2),
                                           in1=g0[:, 4, :], op0=ADD, op1=MAX)
            nc.vector.scalar_tensor_tensor(out=t67[:], in0=g0[:, 7, :], scalar=mask(h, 0, 3),
                                           in1=g0[:, 6, :], op0=ADD, op1=MAX)
            nc.vector.scalar_tensor_tensor(out=t01[:], in0=t23[:], scalar=mask(h, 0, 4),
                                           in1=t01[:], op0=ADD, op1=MAX)
            nc.vector.scalar_tensor_tensor(out=t45[:], in0=t67[:], scalar=mask(h, 0, 5),
                                           in1=t45[:], op0=ADD, op1=MAX)
            nc.vector.scalar_tensor_tensor(out=sh[:, 0, :], in0=t45[:], scalar=mask(h, 0, 6),
                                           in1=t01[:], op0=ADD, op1=MAX)
            # cc1: depth-3 tree on cc1_rows rows (DVE too; walrus rejects
            # DVE-lowered ops on the Pool engine)
            g1 = gn[h][:].rearrange("p one (r d) -> p (one r) d", r=cc1_rows)
            u01 = work.tile([128, D], bf, name=f"u01_{h}", tag=f"u01_{h}")
            u23 = work.tile([128, D], bf, name=f"u23_{h}", tag=f"u23_{h}")
            nc.vector.scalar_tensor_tensor(out=u01[:], in0=g1[:, 1, :], scalar=mask(h, 1, 0),
                                           in1=g1[:, 0, :], op0=ADD, op1=MAX)
            nc.vector.scalar_tensor_tensor(out=u23[:], in0=g1[:, 3, :], scalar=mask(h, 1, 1),
                                           in1=g1[:, 2, :], op0=ADD, op1=MAX)
            if cc1_rows == 5:
                nc.vector.scalar_tensor_tensor(out=u01[:], in0=u23[:], scalar=mask(h, 1, 4),
                                               in1=u01[:], op0=ADD, op1=MAX)
                nc.vector.scalar_tensor_tensor(out=sh[:, 1, :], in0=g1[:, 4, :], scalar=mask(h, 1, 6),
                                               in1=u01[:], op0=ADD, op1=MAX)
            else:
                u45 = work.tile([128, D], bf, name=f"u45_{h}", tag=f"u45_{h}")
                u67 = work.tile([128, D], bf, name=f"u67_{h}", tag=f"u67_{h}")
                nc.vector.scalar_tensor_tensor(out=u45[:], in0=g1[:, 5, :], scalar=mask(h, 1, 2),
                                               in1=g1[:, 4, :], op0=ADD, op1=MAX)
                nc.vector.scalar_tensor_tensor(out=u67[:], in0=g1[:, 7, :], scalar=mask(h, 1, 3),
                                               in1=g1[:, 6, :], op0=ADD, op1=MAX)
                nc.vector.scalar_tensor_tensor(out=u01[:], in0=u23[:], scalar=mask(h, 1, 4),
                                               in1=u01[:], op0=ADD, op1=MAX)
                nc.vector.scalar_tensor_tensor(out=u45[:], in0=u67[:], scalar=mask(h, 1, 5),
                                               in1=u45[:], op0=ADD, op1=MAX)
                nc.vector.scalar_tensor_tensor(out=sh[:, 1, :], in0=u45[:], scalar=mask(h, 1, 6),
                                               in1=u01[:], op0=ADD, op1=MAX)
            sem_b.append(sh)

        def w1c(m):  # W1 chunk m as [128, HID]
            return wpk_t[:, m * HID : (m + 1) * HID]

        def w2c(k):
            return wpk_t[:, WK_W2 + k * REL : WK_W2 + (k + 1) * REL]

        def hsel(b):
            return hts_t[:, b * R : (b + 1) * R]

        def tsel(b):
            return hts_t[:, (NB + b) * R : (NB + b + 1) * R]

        ident = pkb_t[:, PK_ID : PK_ID + 128]

        # ---- per-batch compute ----
        out_sb = work.tile([128, NB, R], bf)
        for b in range(NB):
            inv = pkf_t[:, FK_INV + b : FK_INV + b + 1]
            # vertex pool: V_emb = P @ span_emb, scaled by 1/cnt
            ps_v = psums.tile([128, D], mybir.dt.float32, space="PSUM",
                              tag="ps_v", bufs=1, name="ps_v")
            for cc in range(2):
                pt = pkb_t[:, PK_PT + (b * 2 + cc) * V : PK_PT + (b * 2 + cc + 1) * V]
                for n0, nsz in ((0, 512), (512, 256)):
                    nc.tensor.matmul(
                        ps_v[:, n0 : n0 + nsz], lhsT=pt,
                        rhs=sem_b[b][:, cc, n0 : n0 + nsz],
                        start=(cc == 0), stop=(cc == 1))
            v_sb = perb.tile([V, D], bf, tag="v_sb")
            nc.scalar.activation(v_sb[:], ps_v[:], AF.Copy, scale=inv)

            # V_emb.T via PE transposes
            vt_sb = perb.tile([128, 6, V], bf, tag="vt_sb")
            for m in range(6):
                ps_t = psums.tile([128, 512], bf, space="PSUM",
                                  tag="sel", bufs=3, name="ps_t")
                nc.tensor.transpose(ps_t[:, :128], v_sb[:, m * 128 : (m + 1) * 128], ident)
                nc.scalar.activation(vt_sb[:, m, :], ps_t[:, :128], AF.Copy)

            # head/tail gather (one-hot matmuls) + product
            head_t = perb.tile([128, 6, R], bf, tag="head_t")
            tail_t = perb.tile([128, 6, R], bf, tag="tail_t")
            prod_t = perb.tile([128, 6, R], bf, tag="prod_t")
            for m in range(6):
                ps_h = psum_tile("ps_h", "sel", 3)
                nc.tensor.matmul(ps_h[:], lhsT=v_sb[:, m * 128 : (m + 1) * 128],
                                 rhs=hsel(b), start=True, stop=True)
                nc.vector.tensor_copy(head_t[:, m, :], ps_h[:])
                ps_t2 = psum_tile("ps_t2", "sel", 3)
                nc.tensor.matmul(ps_t2[:], lhsT=v_sb[:, m * 128 : (m + 1) * 128],
                                 rhs=tsel(b), start=True, stop=True)
                nc.scalar.activation(tail_t[:, m, :], ps_t2[:], AF.Copy)
                nc.vector.tensor_tensor(out=prod_t[:, m, :], in0=head_t[:, m, :],
                                        in1=tail_t[:, m, :], op=MUL)

            # Vw_a / Vw_c = (V_emb @ W1a/c) * inv
            vw_a = perb.tile([V, HID], bf, tag="vw_a")
            vw_c = perb.tile([V, HID], bf, tag="vw_c")
            for vw, c0 in ((vw_a, 0), (vw_c, 6)):
                ps_vw = psum_tile("ps_vw", "sel", 3)
                for m in range(6):
                    nc.tensor.matmul(ps_vw[:, :HID], lhsT=vt_sb[:, m, :],
                                     rhs=w1c(c0 + m), start=(m == 0), stop=(m == 5))
                nc.scalar.activation(vw[:], ps_vw[:, :HID], AF.Copy)

            # hidden = relu(sum of five blocks), transposed [HID, R]
            hid_t = perb.tile([128, 3, R], bf, tag="hid_t")
            for m3 in range(3):
                msl = slice(m3 * 128, (m3 + 1) * 128)
                chunks = [(vw_a[:, msl], hsel(b)), (vw_c[:, msl], tsel(b)),
                          (pkb_t[:40, PK_EW + m3 * 128 : PK_EW + (m3 + 1) * 128],
                           pkb_t[:40, PK_ES + b * R : PK_ES + (b + 1) * R])]
                chunks += [(w1c(12 + m)[:, msl], prod_t[:, m, :]) for m in range(6)]
                ps_hid = psum_tile("ps_hid", "hid", 2)
                for i, (lhsT, rhs_ap) in enumerate(chunks):
                    nc.tensor.matmul(ps_hid[:], lhsT=lhsT, rhs=rhs_ap,
                                     start=(i == 0), stop=(i == len(chunks) - 1))
                nc.scalar.activation(hid_t[:, m3, :], ps_hid[:], AF.Relu)

            # out = W2.T @ hid + b2
            ps_o = psum_tile("ps_o", "out", 1)
            for kc in range(W2C):
                nc.tensor.matmul(ps_o[:REL, :], lhsT=w2c(kc), rhs=hid_t[:, kc, :],
                                 start=(kc == 0), stop=(kc == W2C - 1))
            nc.scalar.activation(out_sb[:REL, b, :], ps_o[:REL, :], AF.Identity,
                                 bias=pkf_t[:REL, FK_B2 : FK_B2 + 1])
            nc.sync.dma_start(out=outd.ap()[:, b, :], in_=out_sb[:REL, b, :])

    nc.compile()
    return nc


def _idx_table(flat):
    """128-descriptor gather index table: [128, 8] int16 wrapped/replicated."""
    return np.tile(flat.astype(np.int16).reshape(8, 16).T, (8, 1))


def _prep_core(c, sentence_repr, esi, vidx, vmask, ht, dis_h, dis_t,
               ew2, wpk_a, b2_f):
    """Per-core input map for batches [c*NB, c*NB+NB). Returns (inputs, cc1_rows)."""
    bs = range(c * NB, c * NB + NB)

    sent = np.empty((SENT_ROWS, D), dtype=BF16)
    for j, b in enumerate(bs):
        sent[j * S : (j + 1) * S] = sentence_repr[b].astype(BF16)

    starts = np.stack([esi[b, :, 0] for b in bs])                 # (NB, NS)
    widths = np.stack([esi[b, :, 1] - esi[b, :, 0] for b in bs])  # (NB, NS)

    # sort spans by width desc; cc0 = widest 128 (8-row), cc1 = rest
    perms, cc1_rows = [], 5
    for h in range(NB):
        p = np.argsort(-widths[h], kind="stable")
        perms.append(p)
        if widths[h][p[128:]].max() > 4:
            cc1_rows = 8

    gidx = np.empty((128, NB * 2 * 8), dtype=np.int16)
    masks = np.zeros((128, NB, 2, 7), dtype=np.float32)
    MJW = [1, 3, 5, 7, 2, 6, 4]   # mask j -> invalid when width < MJW[j]
    for h in range(NB):
        st_s = starts[h][perms[h]] + h * S
        w_s = widths[h][perms[h]]
        gidx[:, (2 * h) * 8 : (2 * h) * 8 + 8] = _idx_table(st_s[:128])
        gidx[:, (2 * h + 1) * 8 : (2 * h + 1) * 8 + 8] = _idx_table(st_s[128:])
        for cc in range(2):
            wq = w_s[cc * 128 : (cc + 1) * 128]
            for j, thr in enumerate(MJW):
                masks[:, h, cc, j] = np.where(wq < thr, np.float32(NEG), 0.0)

    pkf = np.zeros((128, FK_N), dtype=np.float32)
    pkf[:, :FK_INV] = masks.reshape(128, -1)
    pkb = np.zeros((128, PK_N), dtype=BF16)
    pkb[:, PK_ID : PK_ID + 128] = np.eye(128, dtype=BF16)
    pkb[:40, PK_EW : PK_EW + HID] = ew2
    hts_a = np.zeros((V, 2 * NB * R), dtype=BF16)
    for j, b in enumerate(bs):
        pt = np.zeros((NS, V), dtype=np.float32)
        np.add.at(pt, (vidx[b].ravel(), np.repeat(np.arange(V), C)),
                  vmask[b].ravel().astype(np.float32))
        pt = pt[perms[j]]                                  # permuted span slots
        pkb[:, PK_PT + (j * 2) * V : PK_PT + (j * 2 + 2) * V] = (
            pt.reshape(2, 128, V).transpose(1, 0, 2).reshape(128, 2 * V).astype(BF16))
        pkf[:, FK_INV + j] = 1.0 / np.maximum(vmask[b].sum(axis=1).astype(np.float32), 1.0)
        hts_a[ht[b, :, 0], j * R + np.arange(R)] = BF16(1.0)
        hts_a[ht[b, :, 1], (NB + j) * R + np.arange(R)] = BF16(1.0)
        es = np.zeros((40, R), dtype=BF16)
        es[dis_h[b], np.arange(R)] = BF16(1.0)
        es[DIS + dis_t[b], np.arange(R)] = BF16(1.0)
        pkb[:40, PK_ES + j * R : PK_ES + (j + 1) * R] = es
    pkf[:REL, FK_B2] = b2_f

    return dict(sent=sent, gidx=gidx, pkf=pkf, pkb=pkb, hts=hts_a, wpk=wpk_a), cc1_rows


def run(trace=False, **inputs):
    sentence_repr = np.asarray(inputs["sentence_repr"], dtype=np.float32)
    esi = np.asarray(inputs["entity_span_indices"]).astype(np.int64)
    vidx = np.asarray(inputs["vertex_indices"]).astype(np.int64)
    vmask = np.asarray(inputs["vertex_indices_mask"]).astype(np.int64)
    ht = np.asarray(inputs["head_tail_indices"]).astype(np.int64)
    dis_h = np.asarray(inputs["dis_h_2_t"]).astype(np.int64)
    dis_t = np.asarray(inputs["dis_t_2_h"]).astype(np.int64)
    dis_embed = np.asarray(inputs["dis_embed"], dtype=np.float32)
    w1 = np.asarray(inputs["W1"], dtype=np.float32)
    w2 = np.asarray(inputs["W2"], dtype=np.float32)
    b2 = np.asarray(inputs["b2"], dtype=np.float32)

    fin = D + DIS
    # host-folded distance-embedding blocks: ew2 = [dis_embed@W1b; dis_embed@W1d]
    ew2 = np.concatenate([dis_embed @ w1[D : D + DIS],
                          dis_embed @ w1[fin + D : fin + D + DIS]], axis=0).astype(BF16)
    # W1 chunks: a = rows 0:768, c = rows 788:1556, e = rows 1576:2344
    w1_abc = np.concatenate([w1[0:D], w1[fin : fin + D], w1[2 * fin : 2 * fin + D]], axis=0)
    w1_p = np.ascontiguousarray(
        w1_abc.astype(BF16).reshape(NW1C, 128, HID).transpose(1, 0, 2)).reshape(128, -1)
    w2_p = np.ascontiguousarray(
        w2.astype(BF16).reshape(W2C, 128, REL).transpose(1, 0, 2)).reshape(128, -1)
    wpk_a = np.concatenate([w1_p, w2_p], axis=1)
    assert wpk_a.shape == (128, WK_N)

    in_maps, cc1_rows = [], 5
    for c in range(NCORES):
        m, cr = _prep_core(c, sentence_repr, esi, vidx, vmask, ht, dis_h, dis_t,
                           ew2, wpk_a, b2)
        in_maps.append(m)
        cc1_rows = max(cc1_rows, cr)

    if cc1_rows not in _NC_CACHE:
        _NC_CACHE[cc1_rows] = _build(cc1_rows)

    res = bass_utils.run_bass_kernel_spmd(
        _NC_CACHE[cc1_rows], in_maps, core_ids=list(range(NCORES)), trace=trace
    )

    out = np.empty((B, R, REL), dtype=np.float32)
    for c in range(NCORES):
        o = np.asarray(res.results[c]["outd"], dtype=np.float32)  # (REL, NB, R)
        for j in range(NB):
            out[c * NB + j] = o[:, j].T
    return out, res


def kernel(**inputs):
    out, _ = run(**inputs)
    return out
